# revision 151
# baseline (speedup 1.0000x reference)
"""Depthwise 13x13 stride-4 conv (AntiAliasInterpolation2d) on 8 TRN2 NeuronCores.

Pure data parallel: batch 32 -> 4 images per core. Two device graphs:

1. rank-1 path (used when each channel's 13x13 kernel is an outer product
   v ⊗ h, which holds for the Gaussian anti-alias kernel): separable conv,
   image-major pipeline, fp8 input stream.

   The images ride to the device as fp8_e4m3 (half the DMA bytes of
   bf16), quantized on the host with error diffusion down image rows so
   the 13-tap vertical Gaussian sees anticorrelated errors (~2x lower
   output error than round-to-nearest). Stage V contracts input rows on
   the TensorEngine with fp8 DoubleRow matmuls (2 contraction rows per
   cycle, 2 row-chunk k-tiles per matmul) against per-chunk 128x128 fp8
   stationaries; the fp8 tap vector v-hat is chosen on the fp8 grid to
   minimize the rank-1 kernel residual (alpha scan + per-tap 1-ulp
   descent) with the compensating rescale folded into the bf16 h taps.
   One PSUM bank per image so each image's V closes as soon as its DMA
   lands; the DVE casts V to bf16 while de-interleaving columns into 4
   phases (image 3 in two halves so the second H group starts sooner).
   Stage H applies the kept horizontal taps (L2-mass threshold drops
   13 -> 9 for the Gaussian) as full-128 diagonal-stationary bf16
   matmuls accumulating in PSUM. Per channel the PE runs all four V
   groups first (the DVE cast chain overlaps them), then the B strip as
   one 4-image group, then both H groups — so H starts stall-free.
   Channel 0's diag stationaries are built on the idle ACT engine
   (identity via affine_select on Pool, scaled by h taps shipped as a
   64-byte prefix); channels 1+2's ride the slack DMA ring between
   image blocks. ACT also drains the B-strip casts, output copies and
   output DMA issue; the sync queue carries only input DMAs, no waits;
   redundant PSUM-bank WAR waits are elided (each satisfied PE wait
   still costs ~250ns of queue time). Dummy warm-up matmuls ramp the
   HAM clock before the stream arrives. PSUM bank discipline: a bank is
   never read by one engine while another engine (or the PE mid-group)
   touches it — concurrent same-bank access faults the hardware.

2. general path (fallback for non-separable weights): direct 2D conv as
   52 PSUM-accumulated banded-Toeplitz bf16 matmuls per channel (13
   kernel columns x 4 row chunks), stride-4 columns de-interleaved on
   the host.

V accumulates in fp32 PSUM; H runs in bf16 on the casted V; output fp32.
Measured rel err 1.58e-2 vs the fp64 reference (budget 2e-2).
"""

import numpy as np
import ml_dtypes

N_CORES = 8
B, C, H, W = 32, 3, 512, 512
KS = 13          # kernel size
PAD = 6          # pad on each side
STR = 4          # stride
OH = OW = 128    # output spatial
PW = W + 2 * PAD  # 524 padded width
NPH = PW // STR   # 131 columns per phase
BPC = B // N_CORES  # images per core = 4
XW = BPC * PW     # 2096 free-dim columns per input tile

# general path epack layout
SLOT = 130
NPAIR = C * KS
EPACK_COLS = (NPAIR - 1) * SLOT + 224

_CACHE = {}
WARMUP_MMS = 7  # pre-stream dummy matmuls to ramp the HAM clock
DVE_WARMUPS = 9  # dummy DVE copies feeding the HAM activity monitor
ACT_WARMUPS = 3   # dummy ACT copies likewise

STCOLS = 512     # per-channel st: 4 explicit 128x128 chunk stationaries
HS = 32          # h-tap scalar columns (one per kept tap, padded to 32)


def _bacc():
    from concourse import bacc

    return bacc.Bacc(
        "TRN2", target_bir_lowering=False, debug=False, num_devices=N_CORES
    )


def _opt_fp8_v(v):
    """fp8-grid v-hat minimizing the rank-1 outer-product residual.

    Scans a global scale alpha, then per-tap +/-1-ulp coordinate descent.
    Returns (v_hat fp8 array, h_scale) with v_hat ⊗ (h*h_scale) ~ v ⊗ h.
    """
    e4 = ml_dtypes.float8_e4m3
    v = np.asarray(v, np.float64)
    vn = v / np.linalg.norm(v)

    def resid(va):
        n = np.linalg.norm(va)
        if n == 0:
            return 1e9
        return np.linalg.norm(va / n - vn)

    best = None
    for alpha in np.linspace(0.75, 1.5, 1501):
        va = (v * alpha).astype(e4).astype(np.float64)
        r = resid(va)
        if best is None or r < best[0]:
            best = (r, va)
    va = best[1].copy()
    # +/- 1 ulp coordinate descent on each tap
    for _ in range(4):
        improved = False
        for i in range(len(va)):
            b = np.float64(va[i])
            for cand in (np.nextafter(e4(b), e4(np.inf)),
                         np.nextafter(e4(b), e4(-np.inf))):
                trial = va.copy()
                trial[i] = np.float64(cand)
                if resid(trial) < resid(va):
                    va = trial
                    improved = True
        if not improved:
            break
    h_scale = float((va * v).sum() / (va * va).sum())
    return va.astype(e4), h_scale


def _build_graph_rank1_raw(keeps=tuple(tuple(range(KS)) for _ in range(C))):
    """Hand-scheduled raw-bacc version: no Tile framework.

    Static buffers: all 3 channels' inputs resident in SBUF (DMAs issued
    back-to-back at t=0), double-buffered V/out staging, 7 PSUM banks
    (4 vertical accumulators + B-strip + 2 horizontal accumulators).
    """
    import concourse.bass as bass  # noqa: F401
    from concourse import mybir
    from contextlib import ExitStack

    nc = _bacc()
    STW = C * STCOLS
    nk = [len(k) for k in keeps]
    hso = [sum(nk[:c]) for c in range(C)]      # per-channel hs col offset
    hdo = [sum(nk[:c]) * 128 for c in range(C)]  # per-channel hd col offset

    f32 = mybir.dt.float32
    bf16 = mybir.dt.bfloat16
    fp8 = mybir.dt.float8e4
    DR = mybir.MatmulPerfMode.DoubleRow
    CW = 4 * XW  # input elems per channel
    X0 = 2 * HS + STW  # byte-columns before the images

    # single fp8 tensor: [hs bytes | st fp8 | fp8 image stream] — half the
    # DMA traffic of the bf16 version, and fp8 stationaries let the V
    # matmuls run in DoubleRow mode (2 contraction rows per cycle)
    x = nc.dram_tensor(
        "x", [128, X0 + 3 * 4 * XW], fp8, kind="ExternalInput"
    )
    # diag H stationaries for channels 1+2 ride the (now slack) DMA ring,
    # slotted between channels; channel 0's are built on ACT (its DMA slot
    # would push channel 0's data-paced V stage out)
    hd = nc.dram_tensor(
        "hd", [128, (nk[1] + nk[2]) * 128], bf16, kind="ExternalInput"
    )
    out = nc.dram_tensor(
        "out", [C, 128, BPC * OW], mybir.dt.bfloat16, kind="ExternalOutput"
    )

    with nc.cleanup_on_exit(), ExitStack() as es:
        xa = es.enter_context(nc.sbuf_tensor("xa", [128, X0 + 3 * CW], fp8))
        hst = xa[:, 0 : 2 * HS].bitcast(bf16)
        stt = xa[:, 2 * HS : X0]
        xt = xa[:, X0 : X0 + 3 * CW]
        hdt = es.enter_context(nc.sbuf_tensor("hdt", [128, sum(nk) * 128], bf16))
        idt = es.enter_context(nc.sbuf_tensor("idt", [128, 128], bf16))
        hsf = es.enter_context(nc.sbuf_tensor("hsf", [128, HS], f32))
        vsb = es.enter_context(nc.sbuf_tensor("vsb", [128, 2 * XW], bf16))
        ot = es.enter_context(nc.sbuf_tensor("ot", [128, 2 * 512], bf16))
        vA = es.enter_context(nc.psum_tensor("vA", [128, 4 * 512], f32))
        wps = es.enter_context(nc.psum_tensor("wps", [128, 512], f32))
        vB = es.enter_context(nc.psum_tensor("vB", [128, 512], f32))
        hp2 = es.enter_context(nc.psum_tensor("hp2", [128, 2 * 512], f32))

        s_x = [
            [es.enter_context(nc.semaphore(f"s_x{c}_{k}")) for k in range(4)]
            for c in range(C)
        ]
        s_hs = es.enter_context(nc.semaphore("s_hs"))
        s_xh0 = es.enter_context(nc.semaphore("s_xh0"))
        s_x0h = [es.enter_context(nc.semaphore(f"s_x0h{g}")) for g in range(4)]
        s_hd = [es.enter_context(nc.semaphore(f"s_hd{c}")) for c in range(C)]
        s_id = es.enter_context(nc.semaphore("s_id"))
        s_mm = [es.enter_context(nc.semaphore(f"s_mm{c}")) for c in range(C)]
        s_vcA = [es.enter_context(nc.semaphore(f"s_vcA{c}")) for c in range(C)]
        s_vcB = [es.enter_context(nc.semaphore(f"s_vcB{c}")) for c in range(C)]
        s_out = [es.enter_context(nc.semaphore(f"s_out{c}")) for c in range(C)]
        s_od = [es.enter_context(nc.semaphore(f"s_od{c}")) for c in range(C)]

        # skip GPSIMD's expensive dge_drain on exit — its queue only runs
        # the tiny identity build, and the drain sits on the critical
        # kernel-end path
        with nc.Block(no_gpsimd_drain=True) as block:

            @block.sync
            def _(sync):
                # one ring, consumption order: the SDMA drains these FIFO
                # at ~line rate, pacing the PE's k-waves. Input only — the
                # output DMAs are issued from the ACT queue.
                def emit_x(c, from_g=0, to_g=4):
                    # image-major: one transfer per image (4 row chunks each)
                    for g in range(from_g, to_g):
                        lo = X0 + c * CW + g * 4 * PW
                        sync.dma_start(
                            xa[:, lo : lo + 4 * PW], x[:, lo : lo + 4 * PW]
                        ).then_inc(s_x[c][g], 16)

                # h taps ride first (8KB: unblocks the on-device stationary
                # builds ASAP), then ONE large transfer with the V
                # stationaries + channel 0's first half-image (a big early
                # transfer streams faster than several small ones); channel
                # 0 streams in half-image pieces so the PE's arrival gaps
                # stay small (keeps the HAM clock ramping). The DMA ring
                # drains transfers in issue order per engine, so a wait on
                # a later transfer implies earlier ones landed.
                half = 2 * PW
                sync.dma_start(
                    xa[:, 0 : 2 * HS], x[:, 0 : 2 * HS]
                ).then_inc(s_hs, 16)
                sync.dma_start(
                    xa[:, 2 * HS : X0 + half], x[:, 2 * HS : X0 + half]
                ).then_inc(s_xh0, 16)
                sync.dma_start(
                    xa[:, X0 + half : X0 + 4 * PW], x[:, X0 + half : X0 + 4 * PW]
                ).then_inc(s_x[0][0], 16)
                for g in range(1, 4):
                    lo = X0 + g * 4 * PW
                    sync.dma_start(
                        xa[:, lo : lo + 4 * PW], x[:, lo : lo + 4 * PW]
                    ).then_inc(s_x0h[g], 16)
                sync.dma_start(
                    hdt[:, hdo[1] : hdo[1] + nk[1] * 128],
                    hd[:, 0 : nk[1] * 128],
                ).then_inc(s_hd[1], 16)
                emit_x(1)
                sync.dma_start(
                    hdt[:, hdo[2] : hdo[2] + nk[2] * 128],
                    hd[:, nk[1] * 128 :],
                ).then_inc(s_hd[2], 16)
                emit_x(2)

            @block.tensor
            def _(tensor):
                def emit_V_img(c, g):
                    # one image: 2 accumulating DoubleRow matmuls (2 row
                    # chunks each, 2 contraction rows per cycle), one bank.
                    # vA/vB bank WARs vs the previous channel's casts are
                    # implied by H(c-1,1)'s gates earlier in program order
                    # (it waits s_vcA>=5 and s_vcB>=2), so no waits here —
                    # each satisfied PE wait still costs ~250ns of queue
                    # time. emit_H asserts those gates were emitted.
                    x0 = c * CW + g * 4 * PW
                    if c >= 1:
                        tensor.wait_ge(s_x[c][g], 16)
                    for kp in range(2):
                        if c == 0:
                            # ring order implies st landed before s_xh0
                            if g == 0:
                                tensor.wait_ge(
                                    s_xh0 if kp == 0 else s_x[0][0], 16
                                )
                            elif kp == 0:
                                tensor.wait_ge(s_x0h[g], 16)
                        lo = c * STCOLS + 2 * kp * 128
                        lhsT = stt[:, lo : lo + 256].rearrange(
                            "p (t m) -> p t m", t=2
                        )
                        rhs = xt[
                            :, x0 + 2 * kp * PW : x0 + 2 * kp * PW + 2 * PW
                        ].rearrange("p (t w) -> p t w", t=2)[:, :, 0:512]
                        mm = tensor.matmul(
                            vA[:, g * 512 : g * 512 + 512],
                            lhsT,
                            rhs,
                            start=(kp == 0),
                            stop=(kp == 1),
                            perf_mode=DR,
                            skip_group_check=True,
                        )
                    mm.then_inc(s_mm[c], 1)

                def emit_B(c, p=None):
                    # B strip: rightmost 12 padded cols; p=None covers all
                    # 4 images in one group (vB WAR implied by H(c-1,*))
                    glo, ghi = (0, BPC) if p is None else (2 * p, 2 * p + 2)
                    xgk = xt[:, c * CW : (c + 1) * CW].rearrange(
                        "p (g k w) -> p k g w", g=BPC, k=4
                    )
                    for kp in range(2):
                        lo = c * STCOLS + 2 * kp * 128
                        lhsT = stt[:, lo : lo + 256].rearrange(
                            "p (t m) -> p t m", t=2
                        )
                        mm = tensor.matmul(
                            vB[:, glo * 12 : ghi * 12],
                            lhsT,
                            xgk[:, 2 * kp : 2 * kp + 2, glo:ghi, 512:524],
                            start=(kp == 0),
                            stop=(kp == 1),
                            perf_mode=DR,
                            skip_group_check=True,
                        )
                    mm.then_inc(s_mm[c], 1)

                def emit_H(c, hf, war=False):
                    # castA counts (s_vcA, DVE): g0=1 g1=2 g2=3 g3a=4 g3b=5.
                    # castB counts (s_vcB, ACT): c0 pair-casts p0=1 p1=2;
                    # c>=1 one all-image cast = 1.
                    b0 = (c % 2) * XW
                    h0 = (c % 2) * 512
                    glo, ghi = 2 * hf, 2 * hf + 2
                    tensor.wait_ge(s_vcA[c], 2 if hf == 0 else 4)
                    tensor.wait_ge(s_hd[c], 1 if c == 0 else 16)
                    if war and c >= 2 and hf == 0:
                        tensor.wait_ge(s_out[c - 2], 1)  # hp bank WAR
                    vg = vsb[:, b0 : b0 + XW].rearrange("p (g w) -> p g w", g=BPC)
                    # later gates arrive incrementally: image 3's second
                    # half-cast, and the B-strip cast that fills cols
                    # 128:131 (needed only by q>=1 taps)
                    vca_need = 5 if hf == 1 else 0
                    # one all-image B cast, needed by q>=1 taps only;
                    # H1 inherits H0's gate via program order
                    vcb_need = 1 if hf == 0 else 0
                    # PSUM accumulation is order-independent, so stream the
                    # taps whose gates arrive first: for H1 the phase 2-3
                    # taps (covered by image 3's first half-cast) go ahead
                    # of the phase 0-1 taps (second half-cast); for H0 the
                    # q=0 taps (no B-strip columns) go first.
                    if hf == 1:
                        order = sorted(keeps[c], key=lambda j: (j % STR < 2))
                    else:
                        order = sorted(keeps[c], key=lambda j: (j // STR >= 1))
                    ntap = len(order)
                    vca_done = 0
                    vcb_done = 0
                    for i, j in enumerate(order):
                        ph, q = j % STR, j // STR
                        if vca_need and hf == 1 and ph < 2 and vca_done < vca_need:
                            tensor.wait_ge(s_vcA[c], vca_need)  # g3b
                            vca_done = vca_need
                        if vcb_need and q >= 1 and vcb_done < vcb_need:
                            tensor.wait_ge(s_vcB[c], vcb_need)  # castB
                            vcb_done = vcb_need
                        off = ph * NPH + q
                        i0 = keeps[c].index(j)  # hdt tiles are in keeps order
                        mm = tensor.matmul(
                            hp2[:, h0 + glo * OW : h0 + ghi * OW],
                            hdt[:, hdo[c] + i0 * 128 : hdo[c] + (i0 + 1) * 128],
                            vg[:, glo:ghi, off : off + OW],
                            start=(i == 0),
                            stop=(i == ntap - 1),
                            skip_group_check=True,
                        )
                    # the V/B emitters rely on these gates for their
                    # prev-channel bank WARs; emit them even if no tap
                    # needed them (possible with unusual kept-tap sets)
                    if vca_need and vca_done < vca_need:
                        tensor.wait_ge(s_vcA[c], vca_need)
                    if vcb_need and vcb_done < vcb_need:
                        tensor.wait_ge(s_vcB[c], vcb_need)
                    mm.then_inc(s_mm[c], 1)

                # HAM warm-up: dummy matmuls on uninitialized SBUF keep
                # the activity monitor busy from the end of the preamble so
                # the real stream starts at full clock instead of K=4/8
                for w in range(WARMUP_MMS):
                    tensor.matmul(
                        wps[:, 0:512],
                        xa[:, 0:128],
                        xa[:, 128:640],
                        start=True,
                        stop=True,
                        skip_group_check=True,
                    )

                def filler(w=128):
                    # closed-group dummy matmul: keeps HAM activity up while
                    # channel 0's stream ramps
                    tensor.matmul(
                        wps[:, 0:w],
                        xa[:, 0:128],
                        xa[:, 128 : 128 + w],
                        start=True,
                        stop=True,
                        skip_group_check=True,
                    )

                # bridge the gap between the warm-ups and the first image's
                # arrival so the HAM activity monitor sees a continuous load
                for w in range(4):
                    filler(256)

                # all V groups first (the DVE cast chain overlaps them, so
                # H starts stall-free), then the B strip as ONE group, then
                # both H groups (s_mm: V0=1 V1=2 V2=3 V3=4 B=5 H0=6 H1=7).
                # NOTE: a PSUM bank must never be read by one engine while
                # the PE still streams into it (same-bank write+read =
                # hardware fault) — that's why B is one group (its cast
                # fires only after the whole group closes) and each
                # channel's output copies after its H1. Channel 0 is
                # data-paced at the throttled clock; fillers keep the HAM
                # activity monitor fed through its image-arrival gaps.
                for c in range(C):
                    for g in range(4):
                        emit_V_img(c, g)
                        # fillers only in the image-ARRIVAL gaps (after
                        # g0-g2); after g3 all data is present and a filler
                        # would just delay B/H on the critical path
                        if c == 0 and g < 3:
                            for w in range(4 if g < 2 else 2):
                                filler(256)
                    emit_B(c)
                    emit_H(c, 0, war=True)
                    emit_H(c, 1)

            @block.vector
            def _(vector):
                # DVE-side warm-up: extra engine activity between the
                # preamble end and the first real cast, in case the HAM
                # clock monitor aggregates across engines
                for w in range(DVE_WARMUPS):
                    vector.tensor_copy(vsb[:, 0:512], vsb[:, 512:1024])

                def castA(c, g):
                    # fp32 PSUM -> bf16 SBUF cast, de-interleaving columns
                    # into 4 phases so H's stride-4 gather is a contiguous
                    # slice in phase space; image 3 in two halves so H(c,1)
                    # starts ~2x sooner after Vg3 closes
                    b0 = (c % 2) * XW
                    vg = vsb[:, b0 : b0 + XW].rearrange(
                        "p (g ph u) -> p g ph u", g=BPC, ph=STR
                    )
                    vector.wait_ge(s_mm[c], g + 1)
                    if c >= 2 and g == 0:
                        vector.wait_ge(s_mm[c - 2], 7)  # vsb WAR vs H(c-2)
                    srcA = vA[:, g * 512 : g * 512 + 512].rearrange(
                        "p (u ph) -> p ph u", ph=STR
                    )
                    if g < 3:
                        vector.tensor_copy(vg[:, g, :, 0:128], srcA).then_inc(
                            s_vcA[c], 1
                        )
                    else:
                        vector.tensor_copy(
                            vg[:, g, 2:4, 0:128], srcA[:, 2:4, :]
                        ).then_inc(s_vcA[c], 1)
                        vector.tensor_copy(
                            vg[:, g, 0:2, 0:128], srcA[:, 0:2, :]
                        ).then_inc(s_vcA[c], 1)

                for c in range(C):
                    for g in range(4):
                        castA(c, g)

            @block.gpsimd
            def _(gpsimd):
                # identity stationary built before the stream arrives
                gpsimd.memset(idt[:], 1.0)
                gpsimd.affine_select(
                    out=idt[:],
                    in_=idt[:],
                    compare_op=mybir.AluOpType.is_equal,
                    fill=0.0,
                    base=0,
                    pattern=[[-1, 128]],
                    channel_multiplier=1,
                ).then_inc(s_id, 1)


            @block.scalar
            def _(scalar):
                # ACT-side warm-up (see DVE warm-up note)
                for w in range(ACT_WARMUPS):
                    scalar.copy(ot[:, 0:512], ot[:, 512:1024])
                # channel 0's diag stationaries: built on ACT while the
                # first image streams in
                scalar.wait_ge(s_hs, 16)
                scalar.copy(hsf[:], hst)  # bf16 -> f32 scalars
                scalar.wait_ge(s_id, 1)
                for i in range(nk[0]):
                    op = scalar.mul(
                        hdt[:, i * 128 : (i + 1) * 128],
                        idt[:],
                        hsf[:, i : i + 1],
                    )
                op.then_inc(s_hd[0], 1)

                def castB(c):
                    # one cast covering the whole 4-image B strip
                    b0 = (c % 2) * XW
                    vg = vsb[:, b0 : b0 + XW].rearrange(
                        "p (g ph u) -> p g ph u", g=BPC, ph=STR
                    )
                    scalar.wait_ge(s_mm[c], 5)
                    if c >= 2:
                        scalar.wait_ge(s_mm[c - 2], 7)  # vsb WAR vs H(c-2)
                    srcB = vB[:, 0 : BPC * 12].rearrange(
                        "p (g u ph) -> p g ph u", g=BPC, ph=STR
                    )
                    scalar.copy(
                        vg[:, 0:BPC, :, 128:131], srcB
                    ).then_inc(s_vcB[c], 1)

                def emit_out(c):
                    # one full copy + one DMA per channel, after its H1
                    # closes (reading an hp2 bank mid-H1 would race the PE)
                    o0 = (c % 2) * 512
                    scalar.wait_ge(s_mm[c], 7)
                    if c >= 2:
                        scalar.wait_ge(s_od[c - 2], 16)  # ot slot WAR
                    scalar.copy(
                        ot[:, o0 : o0 + 512], hp2[:, o0 : o0 + 512]
                    ).then_inc(s_out[c], 1)
                    scalar.dma_start(
                        out[c][:, 0:512], ot[:, o0 : o0 + 512]
                    ).then_inc(s_od[c], 16)

                for c in range(C):
                    castB(c)
                    emit_out(c)

        # (Block.__exit__ already emits the final barrier; an explicit
        # all_engine_barrier here would re-add the skipped gpsimd drain)
    nc.finalize()
    return nc


def _build_graph_general():
    import concourse.tile as tile
    from concourse import mybir

    nc = _bacc()
    x = nc.dram_tensor("x", [C, 4, 128, XW], mybir.dt.bfloat16, kind="ExternalInput")
    ep = nc.dram_tensor("ep", [128, EPACK_COLS], mybir.dt.bfloat16, kind="ExternalInput")
    out = nc.dram_tensor("out", [BPC, C, OH, OW], mybir.dt.float32, kind="ExternalOutput")

    with tile.TileContext(nc) as tc:
        with (
            tc.tile_pool(name="const", bufs=1) as constp,
            tc.tile_pool(name="xin", bufs=4) as xin,
            tc.tile_pool(name="ps", bufs=2, space="PSUM") as psp,
            tc.tile_pool(name="ot", bufs=2) as otp,
        ):
            ept = constp.tile([128, EPACK_COLS], mybir.dt.bfloat16)
            nc.scalar.dma_start(ept[:], ep[:])
            for c in range(C):
                psum = psp.tile([128, BPC * OW], mybir.dt.float32)
                for k in range(4):
                    xt = xin.tile([128, XW], mybir.dt.bfloat16)
                    nc.sync.dma_start(xt[:], x[c, k])
                    xg = xt[:].rearrange("p (g w) -> p g w", g=BPC)
                    for j in range(KS):
                        ph, q = j % STR, j // STR
                        off = ph * NPH + q
                        rhs = xg[:, :, off : off + OW]
                        t = c * KS + j
                        lo = t * SLOT + 96 - 32 * k
                        lhsT = ept[:, lo : lo + 128]
                        nc.tensor.matmul(
                            psum[:],
                            lhsT,
                            rhs,
                            start=(k == 0 and j == 0),
                            stop=(k == 3 and j == KS - 1),
                        )
                o = otp.tile([128, BPC * OW], mybir.dt.float32)
                nc.vector.tensor_copy(o[:], psum[:])
                dst = out[:, c].rearrange("g y x -> y g x")
                nc.sync.dma_start(dst, o[:].rearrange("y (g x) -> y g x", g=BPC))
    nc.finalize()
    return nc


def _decompose(weight):
    """Per-channel SVD; return (v[c,13], h[c,13]) if rank-1, else None."""
    vs, hs = [], []
    for c in range(C):
        w = weight[c, 0].astype(np.float64)
        u, s, vt = np.linalg.svd(w)
        if s[1] > 1e-5 * s[0]:
            return None
        sc = np.sqrt(s[0])
        vs.append(u[:, 0] * sc)
        hs.append(vt[0] * sc)
    return np.stack(vs), np.stack(hs)


def _pad_shard(inp):
    """[32,3,512,512] f32 -> [core, c, 128, k*img*524] fp8 (padded cols).

    fp8_e4m3 with error diffusion down image rows: quantization error of
    row r is carried into row r+1 before quantizing it, so the 13-tap
    vertical Gaussian sum sees anticorrelated errors (~2x lower output
    error than round-to-nearest fp8).
    """
    e4 = ml_dtypes.float8_e4m3
    pad = np.zeros((B, C, H, PW), np.float32)
    pad[..., PAD : PAD + W] = inp
    q = np.empty((B, C, H, PW), e4)
    carry = np.zeros((B, C, PW), np.float32)
    for r in range(H):
        t = pad[:, :, r, :] + carry
        qr = t.astype(e4)
        carry = t - qr.astype(np.float32)
        q[:, :, r, :] = qr
    arr = q.reshape(N_CORES, BPC, C, 4, 128, PW)
    arr = arr.transpose(0, 2, 4, 1, 3, 5).reshape(N_CORES, C, 128, 4 * XW)
    return np.ascontiguousarray(arr)


def _phase_shard(inp):
    """[32,3,512,512] f32 -> padded + phase-deinterleaved shards (general)."""
    bf16 = ml_dtypes.bfloat16
    pad = np.zeros((B, C, H, PW), np.float32)
    pad[..., PAD : PAD + W] = inp
    phmat = pad.reshape(B, C, H, NPH, STR).transpose(0, 1, 2, 4, 3)
    arr = phmat.reshape(N_CORES, BPC, C, 4, 128, STR, NPH)
    arr = arr.transpose(0, 2, 3, 4, 1, 5, 6).reshape(N_CORES, C, 4, 128, XW)
    return np.ascontiguousarray(arr).astype(bf16)


def _prep_rank1(inp, v, h):
    bf16 = ml_dtypes.bfloat16
    e4 = ml_dtypes.float8_e4m3
    arr = _pad_shard(inp)
    # fp8 chunk stationaries (DoubleRow matmuls need both operands fp8);
    # v-hat is chosen on the fp8 grid to minimize the rank-1 kernel
    # residual, with the compensating rescale folded into the h taps
    st = np.zeros((C, 4, 128, 128), np.float32)
    hadj = np.zeros((C, KS), np.float64)
    rr = np.arange(128)[:, None]   # contraction row within chunk
    mo = np.arange(128)[None, :]   # output row
    for c in range(C):
        vq, h_scale = _opt_fp8_v(v[c])
        vqf = vq.astype(np.float64)
        hadj[c] = np.asarray(h[c], np.float64) * h_scale
        for k in range(4):
            tap = rr + 128 * k - 4 * mo + PAD
            m = (tap >= 0) & (tap < KS)
            E = np.zeros((128, 128), np.float32)
            E[m] = vqf[tap[m]]
            st[c, k] = E
    st = np.ascontiguousarray(
        st.transpose(2, 0, 1, 3).reshape(128, C * STCOLS)
    ).astype(e4)
    # adaptive tap dropping: discard the largest set of horizontal taps
    # whose combined L2 mass is <= 5e-3 of the tap vector's norm (adds
    # ~3e-3 output rel err, far under the fp8 noise floor already present)
    keeps = []
    for c in range(C):
        a = np.abs(hadj[c])
        order = np.argsort(a)
        csq = np.cumsum(a[order] ** 2)
        ndrop = int(np.searchsorted(csq, (5e-3 * np.linalg.norm(hadj[c])) ** 2, "right"))
        ndrop = min(ndrop, KS - 1)  # always keep at least one tap
        keep = tuple(sorted(order[ndrop:].tolist()))
        keeps.append(keep)
    keeps = tuple(keeps)
    # kept h taps, replicated down 128 partitions: the device scales an
    # on-device identity by these to form the diag H stationaries
    nk = [len(k) for k in keeps]
    hsm = np.zeros((128, HS), np.float32)
    o = 0
    for c in range(C):
        for j in keeps[c]:
            hsm[:, o] = hadj[c, j]
            o += 1
    hsm = hsm.astype(bf16)
    # channels 1+2's diag H stationaries ride the DMA ring mid-stream
    hdm = np.zeros((128, (nk[1] + nk[2]) * 128), np.float32)
    idx = np.arange(128)
    o = 0
    for c in (1, 2):
        for j in keeps[c]:
            hdm[idx, o + idx] = hadj[c, j]
            o += 128
    hdm = hdm.astype(bf16)
    # h taps (bf16 bytes) + fp8 stationaries ride in front of the stream
    prefix = np.concatenate(
        [np.ascontiguousarray(hsm).view(np.uint8),
         np.ascontiguousarray(st).view(np.uint8)], axis=1
    )
    return keeps, [
        {
            "x": np.ascontiguousarray(
                np.concatenate(
                    [prefix,
                     arr[core].transpose(1, 0, 2)
                     .reshape(128, 3 * 4 * XW).view(np.uint8)],
                    axis=1,
                )
            ).view(e4),
            "hd": hdm,
        }
        for core in range(N_CORES)
    ]


def _prep_general(inp, weight):
    bf16 = ml_dtypes.bfloat16
    arr = _phase_shard(inp)
    epk = np.zeros((128, EPACK_COLS), np.float32)
    r = np.arange(128)
    for c in range(C):
        for j in range(KS):
            t = c * KS + j
            for s in range(-2, 34):
                i = r - 4 * s + PAD
                m = (i >= 0) & (i < KS)
                if m.any():
                    epk[m, t * SLOT + 96 + s] = weight[c, 0, i[m], j]
    epk = epk.astype(bf16)
    return [{"x": arr[core], "ep": epk} for core in range(N_CORES)]


def _prep(inp, weight):
    """Returns (graph_key, in_maps)."""
    inp = np.asarray(inp, dtype=np.float32)
    weight = np.asarray(weight, dtype=np.float32)
    vh = _decompose(weight)
    if vh is not None:
        keeps, in_maps = _prep_rank1(inp, *vh)
        return ("rank1", keeps), in_maps
    return "general", _prep_general(inp, weight)


def _graph(key):
    if key not in _CACHE:
        if key == "general":
            _CACHE[key] = _build_graph_general()
        else:
            _CACHE[key] = _build_graph_rank1_raw(key[1])
    return _CACHE[key]


def _run(key, in_maps):
    from concourse.bass_utils import run_bass_kernel_spmd

    nc = _graph(key)
    res = run_bass_kernel_spmd(nc, in_maps, core_ids=list(range(N_CORES)))
    outs = []
    for i in range(N_CORES):
        o = np.asarray(res.results[i]["out"])
        if o.ndim == 3:  # rank1 layout [C, OH, BPC*OW] -> [BPC, C, OH, OW]
            o = o.reshape(C, OH, BPC, OW).transpose(2, 0, 1, 3)
        outs.append(o)
    return np.concatenate(outs, axis=0).astype(np.float32)


def kernel(inp, weight):
    inp = np.asarray(inp, dtype=np.float32)
    weight = np.asarray(weight, dtype=np.float32)
    key, in_maps = _prep(inp, weight)
    try:
        return _run(key, in_maps)
    except Exception:
        if key == "general":
            raise
        # fall back to the general (weight-agnostic) graph
        return _run("general", _prep_general(inp, weight))


# revision 152
# speedup vs baseline: 1.0087x; 1.0087x over previous
"""Depthwise 13x13 stride-4 conv (AntiAliasInterpolation2d) on 8 TRN2 NeuronCores.

Pure data parallel: batch 32 -> 4 images per core. Two device graphs:

1. rank-1 path (used when each channel's 13x13 kernel is an outer product
   v ⊗ h, which holds for the Gaussian anti-alias kernel): separable conv,
   image-major pipeline, fp8 input stream.

   The images ride to the device as fp8_e4m3 (half the DMA bytes of
   bf16), quantized on the host with error diffusion down image rows so
   the 13-tap vertical Gaussian sees anticorrelated errors (~2x lower
   output error than round-to-nearest). Stage V contracts input rows on
   the TensorEngine with fp8 DoubleRow matmuls (2 contraction rows per
   cycle, 2 row-chunk k-tiles per matmul) against per-chunk 128x128 fp8
   stationaries; the fp8 tap vector v-hat is chosen on the fp8 grid to
   minimize the rank-1 kernel residual (alpha scan + per-tap 1-ulp
   descent) with the compensating rescale folded into the bf16 h taps.
   One PSUM bank per image so each image's V closes as soon as its DMA
   lands; the DVE casts V to bf16 while de-interleaving columns into 4
   phases (image 3 in two halves so the second H group starts sooner).
   Stage H applies the kept horizontal taps (L2-mass threshold drops
   13 -> 9 for the Gaussian) as full-128 diagonal-stationary bf16
   matmuls accumulating in PSUM. Per channel the PE runs all four V
   groups first (the DVE cast chain overlaps them), then the B strip as
   one 4-image group, then both H groups — so H starts stall-free.
   Channel 0's diag stationaries are built on the idle ACT engine
   (identity via affine_select on Pool, scaled by h taps shipped as a
   64-byte prefix); channels 1+2's ride the slack DMA ring between
   image blocks. ACT also drains the B-strip casts, output copies and
   output DMA issue; the sync queue carries only input DMAs, no waits;
   redundant PSUM-bank WAR waits are elided (each satisfied PE wait
   still costs ~250ns of queue time). Dummy warm-up matmuls ramp the
   HAM clock before the stream arrives. PSUM bank discipline: a bank is
   never read by one engine while another engine (or the PE mid-group)
   touches it — concurrent same-bank access faults the hardware.

2. general path (fallback for non-separable weights): direct 2D conv as
   52 PSUM-accumulated banded-Toeplitz bf16 matmuls per channel (13
   kernel columns x 4 row chunks), stride-4 columns de-interleaved on
   the host.

V accumulates in fp32 PSUM; H runs in bf16 on the casted V; output fp32.
Measured rel err 1.58e-2 vs the fp64 reference (budget 2e-2).
"""

import numpy as np
import ml_dtypes

N_CORES = 8
B, C, H, W = 32, 3, 512, 512
KS = 13          # kernel size
PAD = 6          # pad on each side
STR = 4          # stride
OH = OW = 128    # output spatial
PW = W + 2 * PAD  # 524 padded width
NPH = PW // STR   # 131 columns per phase
BPC = B // N_CORES  # images per core = 4
XW = BPC * PW     # 2096 free-dim columns per input tile

# general path epack layout
SLOT = 130
NPAIR = C * KS
EPACK_COLS = (NPAIR - 1) * SLOT + 224

_CACHE = {}
WARMUP_MMS = 7  # pre-stream dummy matmuls to ramp the HAM clock
DVE_WARMUPS = 9  # dummy DVE copies feeding the HAM activity monitor
ACT_WARMUPS = 3   # dummy ACT copies likewise

STCOLS = 512     # per-channel st: 4 explicit 128x128 chunk stationaries
HS = 32          # h-tap scalar columns (one per kept tap, padded to 32)


def _bacc():
    from concourse import bacc

    return bacc.Bacc(
        "TRN2", target_bir_lowering=False, debug=False, num_devices=N_CORES
    )


def _opt_fp8_v(v):
    """fp8-grid v-hat minimizing the rank-1 outer-product residual.

    Scans a global scale alpha, then per-tap +/-1-ulp coordinate descent.
    Returns (v_hat fp8 array, h_scale) with v_hat ⊗ (h*h_scale) ~ v ⊗ h.
    """
    e4 = ml_dtypes.float8_e4m3
    v = np.asarray(v, np.float64)
    vn = v / np.linalg.norm(v)

    def resid(va):
        n = np.linalg.norm(va)
        if n == 0:
            return 1e9
        return np.linalg.norm(va / n - vn)

    best = None
    for alpha in np.linspace(0.75, 1.5, 1501):
        va = (v * alpha).astype(e4).astype(np.float64)
        r = resid(va)
        if best is None or r < best[0]:
            best = (r, va)
    va = best[1].copy()
    # +/- 1 ulp coordinate descent on each tap
    for _ in range(4):
        improved = False
        for i in range(len(va)):
            b = np.float64(va[i])
            for cand in (np.nextafter(e4(b), e4(np.inf)),
                         np.nextafter(e4(b), e4(-np.inf))):
                trial = va.copy()
                trial[i] = np.float64(cand)
                if resid(trial) < resid(va):
                    va = trial
                    improved = True
        if not improved:
            break
    h_scale = float((va * v).sum() / (va * va).sum())
    return va.astype(e4), h_scale


def _build_graph_rank1_raw(keeps=tuple(tuple(range(KS)) for _ in range(C))):
    """Hand-scheduled raw-bacc version: no Tile framework.

    Static buffers: all 3 channels' inputs resident in SBUF (DMAs issued
    back-to-back at t=0), double-buffered V/out staging, 7 PSUM banks
    (4 vertical accumulators + B-strip + 2 horizontal accumulators).
    """
    import concourse.bass as bass  # noqa: F401
    from concourse import mybir
    from contextlib import ExitStack

    nc = _bacc()
    STW = C * STCOLS
    nk = [len(k) for k in keeps]
    hso = [sum(nk[:c]) for c in range(C)]      # per-channel hs col offset
    hdo = [sum(nk[:c]) * 128 for c in range(C)]  # per-channel hd col offset

    f32 = mybir.dt.float32
    bf16 = mybir.dt.bfloat16
    fp8 = mybir.dt.float8e4
    DR = mybir.MatmulPerfMode.DoubleRow
    CW = 4 * XW  # input elems per channel
    X0 = 2 * HS + STW  # byte-columns before the images

    # single fp8 tensor: [hs bytes | st fp8 | fp8 image stream] — half the
    # DMA traffic of the bf16 version, and fp8 stationaries let the V
    # matmuls run in DoubleRow mode (2 contraction rows per cycle)
    x = nc.dram_tensor(
        "x", [128, X0 + 3 * 4 * XW], fp8, kind="ExternalInput"
    )
    # diag H stationaries for channels 1+2 ride the (now slack) DMA ring,
    # slotted between channels; channel 0's are built on ACT (its DMA slot
    # would push channel 0's data-paced V stage out)
    hd = nc.dram_tensor(
        "hd", [128, (nk[1] + nk[2]) * 128], bf16, kind="ExternalInput"
    )
    out = nc.dram_tensor(
        "out", [C, 128, BPC * OW], mybir.dt.bfloat16, kind="ExternalOutput"
    )

    with nc.cleanup_on_exit(), ExitStack() as es:
        xa = es.enter_context(nc.sbuf_tensor("xa", [128, X0 + 3 * CW], fp8))
        hst = xa[:, 0 : 2 * HS].bitcast(bf16)
        stt = xa[:, 2 * HS : X0]
        xt = xa[:, X0 : X0 + 3 * CW]
        hdt = es.enter_context(nc.sbuf_tensor("hdt", [128, sum(nk) * 128], bf16))
        idt = es.enter_context(nc.sbuf_tensor("idt", [128, 128], bf16))
        hsf = es.enter_context(nc.sbuf_tensor("hsf", [128, HS], f32))
        vsb = es.enter_context(nc.sbuf_tensor("vsb", [128, 2 * XW], bf16))
        ot = es.enter_context(nc.sbuf_tensor("ot", [128, 2 * 512], bf16))
        vA = es.enter_context(nc.psum_tensor("vA", [128, 4 * 512], f32))
        wps = es.enter_context(nc.psum_tensor("wps", [128, 512], f32))
        vB = es.enter_context(nc.psum_tensor("vB", [128, 512], f32))
        hp2 = es.enter_context(nc.psum_tensor("hp2", [128, 2 * 512], f32))

        s_x = [
            [es.enter_context(nc.semaphore(f"s_x{c}_{k}")) for k in range(4)]
            for c in range(C)
        ]
        s_hs = es.enter_context(nc.semaphore("s_hs"))
        s_xh0 = es.enter_context(nc.semaphore("s_xh0"))
        s_x0h = [es.enter_context(nc.semaphore(f"s_x0h{g}")) for g in range(4)]
        s_hd = [es.enter_context(nc.semaphore(f"s_hd{c}")) for c in range(C)]
        s_id = es.enter_context(nc.semaphore("s_id"))
        s_mm = [es.enter_context(nc.semaphore(f"s_mm{c}")) for c in range(C)]
        s_vcA = [es.enter_context(nc.semaphore(f"s_vcA{c}")) for c in range(C)]
        s_vcB = [es.enter_context(nc.semaphore(f"s_vcB{c}")) for c in range(C)]
        s_out = [es.enter_context(nc.semaphore(f"s_out{c}")) for c in range(C)]
        s_od = [es.enter_context(nc.semaphore(f"s_od{c}")) for c in range(C)]

        # skip GPSIMD's expensive dge_drain on exit — its queue only runs
        # the tiny identity build, and the drain sits on the critical
        # kernel-end path
        with nc.Block(no_gpsimd_drain=True) as block:

            @block.sync
            def _(sync):
                # one ring, consumption order: the SDMA drains these FIFO
                # at ~line rate, pacing the PE's k-waves. Input only — the
                # output DMAs are issued from the ACT queue.
                def emit_x(c, from_g=0, to_g=4):
                    # image-major: one transfer per image (4 row chunks each)
                    for g in range(from_g, to_g):
                        lo = X0 + c * CW + g * 4 * PW
                        sync.dma_start(
                            xa[:, lo : lo + 4 * PW], x[:, lo : lo + 4 * PW]
                        ).then_inc(s_x[c][g], 16)

                # h taps ride first (8KB: unblocks the on-device stationary
                # builds ASAP), then ONE large transfer with the V
                # stationaries + channel 0's first half-image (a big early
                # transfer streams faster than several small ones); channel
                # 0 streams in half-image pieces so the PE's arrival gaps
                # stay small (keeps the HAM clock ramping). The DMA ring
                # drains transfers in issue order per engine, so a wait on
                # a later transfer implies earlier ones landed.
                half = 2 * PW
                sync.dma_start(
                    xa[:, 0 : 2 * HS], x[:, 0 : 2 * HS]
                ).then_inc(s_hs, 16)
                sync.dma_start(
                    xa[:, 2 * HS : X0 + half], x[:, 2 * HS : X0 + half]
                ).then_inc(s_xh0, 16)
                sync.dma_start(
                    xa[:, X0 + half : X0 + 4 * PW], x[:, X0 + half : X0 + 4 * PW]
                ).then_inc(s_x[0][0], 16)
                for g in range(1, 4):
                    lo = X0 + g * 4 * PW
                    sync.dma_start(
                        xa[:, lo : lo + 4 * PW], x[:, lo : lo + 4 * PW]
                    ).then_inc(s_x0h[g], 16)
                sync.dma_start(
                    hdt[:, hdo[1] : hdo[1] + nk[1] * 128],
                    hd[:, 0 : nk[1] * 128],
                ).then_inc(s_hd[1], 16)
                emit_x(1)
                sync.dma_start(
                    hdt[:, hdo[2] : hdo[2] + nk[2] * 128],
                    hd[:, nk[1] * 128 :],
                ).then_inc(s_hd[2], 16)
                emit_x(2)

            @block.tensor
            def _(tensor):
                def emit_V_img(c, g):
                    # one image: 2 accumulating DoubleRow matmuls (2 row
                    # chunks each, 2 contraction rows per cycle), one bank.
                    # vA/vB bank WARs vs the previous channel's casts are
                    # implied by H(c-1,1)'s gates earlier in program order
                    # (it waits s_vcA>=5 and s_vcB>=2), so no waits here —
                    # each satisfied PE wait still costs ~250ns of queue
                    # time. emit_H asserts those gates were emitted.
                    x0 = c * CW + g * 4 * PW
                    if c >= 1:
                        tensor.wait_ge(s_x[c][g], 16)
                    for kp in range(2):
                        if c == 0:
                            # ring order implies st landed before s_xh0
                            if g == 0:
                                tensor.wait_ge(
                                    s_xh0 if kp == 0 else s_x[0][0], 16
                                )
                            elif kp == 0:
                                tensor.wait_ge(s_x0h[g], 16)
                        lo = c * STCOLS + 2 * kp * 128
                        lhsT = stt[:, lo : lo + 256].rearrange(
                            "p (t m) -> p t m", t=2
                        )
                        rhs = xt[
                            :, x0 + 2 * kp * PW : x0 + 2 * kp * PW + 2 * PW
                        ].rearrange("p (t w) -> p t w", t=2)[:, :, 0:512]
                        mm = tensor.matmul(
                            vA[:, g * 512 : g * 512 + 512],
                            lhsT,
                            rhs,
                            start=(kp == 0),
                            stop=(kp == 1),
                            perf_mode=DR,
                            skip_group_check=True,
                        )
                    mm.then_inc(s_mm[c], 1)

                def emit_B(c, p=None):
                    # B strip: rightmost 12 padded cols; p=None covers all
                    # 4 images in one group (vB WAR implied by H(c-1,*))
                    glo, ghi = (0, BPC) if p is None else (2 * p, 2 * p + 2)
                    xgk = xt[:, c * CW : (c + 1) * CW].rearrange(
                        "p (g k w) -> p k g w", g=BPC, k=4
                    )
                    for kp in range(2):
                        lo = c * STCOLS + 2 * kp * 128
                        lhsT = stt[:, lo : lo + 256].rearrange(
                            "p (t m) -> p t m", t=2
                        )
                        mm = tensor.matmul(
                            vB[:, glo * 12 : ghi * 12],
                            lhsT,
                            xgk[:, 2 * kp : 2 * kp + 2, glo:ghi, 512:524],
                            start=(kp == 0),
                            stop=(kp == 1),
                            perf_mode=DR,
                            skip_group_check=True,
                        )
                    mm.then_inc(s_mm[c], 1)

                def emit_H(c, hf, war=False):
                    # castA counts (s_vcA, DVE): g0=1 g1=2 g2=3 g3a=4 g3b=5.
                    # castB counts (s_vcB, ACT): c0 pair-casts p0=1 p1=2;
                    # c>=1 one all-image cast = 1.
                    b0 = (c % 2) * XW
                    h0 = (c % 2) * 512
                    glo, ghi = 2 * hf, 2 * hf + 2
                    tensor.wait_ge(s_vcA[c], 2 if hf == 0 else 4)
                    tensor.wait_ge(s_hd[c], 1 if c == 0 else 16)
                    if war and c >= 2 and hf == 0:
                        tensor.wait_ge(s_out[c - 2], 1)  # hp bank WAR
                    vg = vsb[:, b0 : b0 + XW].rearrange("p (g w) -> p g w", g=BPC)
                    # later gates arrive incrementally: image 3's second
                    # half-cast, and the B-strip cast that fills cols
                    # 128:131 (needed only by q>=1 taps)
                    vca_need = 5 if hf == 1 else 0
                    # one all-image B cast, needed by q>=1 taps only;
                    # H1 inherits H0's gate via program order
                    vcb_need = 1 if hf == 0 else 0
                    # PSUM accumulation is order-independent, so stream the
                    # taps whose gates arrive first: for H1 the phase 2-3
                    # taps (covered by image 3's first half-cast) go ahead
                    # of the phase 0-1 taps (second half-cast); for H0 the
                    # q=0 taps (no B-strip columns) go first.
                    if hf == 1:
                        order = sorted(keeps[c], key=lambda j: (j % STR < 2))
                    else:
                        order = sorted(keeps[c], key=lambda j: (j // STR >= 1))
                    ntap = len(order)
                    vca_done = 0
                    vcb_done = 0
                    for i, j in enumerate(order):
                        ph, q = j % STR, j // STR
                        if vca_need and hf == 1 and ph < 2 and vca_done < vca_need:
                            tensor.wait_ge(s_vcA[c], vca_need)  # g3b
                            vca_done = vca_need
                        if vcb_need and q >= 1 and vcb_done < vcb_need:
                            tensor.wait_ge(s_vcB[c], vcb_need)  # castB
                            vcb_done = vcb_need
                        off = ph * NPH + q
                        i0 = keeps[c].index(j)  # hdt tiles are in keeps order
                        mm = tensor.matmul(
                            hp2[:, h0 + glo * OW : h0 + ghi * OW],
                            hdt[:, hdo[c] + i0 * 128 : hdo[c] + (i0 + 1) * 128],
                            vg[:, glo:ghi, off : off + OW],
                            start=(i == 0),
                            stop=(i == ntap - 1),
                            skip_group_check=True,
                        )
                    # the V/B emitters rely on these gates for their
                    # prev-channel bank WARs; emit them even if no tap
                    # needed them (possible with unusual kept-tap sets)
                    if vca_need and vca_done < vca_need:
                        tensor.wait_ge(s_vcA[c], vca_need)
                    if vcb_need and vcb_done < vcb_need:
                        tensor.wait_ge(s_vcB[c], vcb_need)
                    mm.then_inc(s_mm[c], 1)

                # HAM warm-up: dummy matmuls on uninitialized SBUF keep
                # the activity monitor busy from the end of the preamble so
                # the real stream starts at full clock instead of K=4/8
                for w in range(WARMUP_MMS):
                    tensor.matmul(
                        wps[:, 0:512],
                        xa[:, 0:128],
                        xa[:, 128:640],
                        start=True,
                        stop=True,
                        skip_group_check=True,
                    )

                def filler(w=128):
                    # closed-group dummy matmul: keeps HAM activity up while
                    # channel 0's stream ramps
                    tensor.matmul(
                        wps[:, 0:w],
                        xa[:, 0:128],
                        xa[:, 128 : 128 + w],
                        start=True,
                        stop=True,
                        skip_group_check=True,
                    )

                # bridge the gap between the warm-ups and the first image's
                # arrival so the HAM activity monitor sees a continuous load
                for w in range(4):
                    filler(256)

                # all V groups first (the DVE cast chain overlaps them, so
                # H starts stall-free), then the B strip as ONE group, then
                # both H groups (s_mm: V0=1 V1=2 V2=3 V3=4 B=5 H0=6 H1=7).
                # NOTE: a PSUM bank must never be read by one engine while
                # the PE still streams into it (same-bank write+read =
                # hardware fault) — that's why B is one group (its cast
                # fires only after the whole group closes) and each
                # channel's output copies after its H1. Channel 0 is
                # data-paced at the throttled clock; fillers keep the HAM
                # activity monitor fed through its image-arrival gaps.
                for c in range(C):
                    for g in range(4):
                        emit_V_img(c, g)
                        # fillers only in the image-ARRIVAL gaps (after
                        # g0-g2); after g3 all data is present and a filler
                        # would just delay B/H on the critical path
                        if c == 0 and g < 3:
                            for w in range(4 if g < 2 else 2):
                                filler(256)
                    emit_B(c)
                    emit_H(c, 0, war=True)
                    emit_H(c, 1)

            @block.vector
            def _(vector):
                # DVE-side warm-up: extra engine activity between the
                # preamble end and the first real cast, in case the HAM
                # clock monitor aggregates across engines
                for w in range(DVE_WARMUPS):
                    vector.tensor_copy(vsb[:, 0:512], vsb[:, 512:1024])

                def castA(c, g):
                    # fp32 PSUM -> bf16 SBUF cast, de-interleaving columns
                    # into 4 phases so H's stride-4 gather is a contiguous
                    # slice in phase space; image 3 in two halves so H(c,1)
                    # starts ~2x sooner after Vg3 closes
                    b0 = (c % 2) * XW
                    vg = vsb[:, b0 : b0 + XW].rearrange(
                        "p (g ph u) -> p g ph u", g=BPC, ph=STR
                    )
                    vector.wait_ge(s_mm[c], g + 1)
                    if c >= 2 and g == 0:
                        vector.wait_ge(s_mm[c - 2], 7)  # vsb WAR vs H(c-2)
                    srcA = vA[:, g * 512 : g * 512 + 512].rearrange(
                        "p (u ph) -> p ph u", ph=STR
                    )
                    if g < 3:
                        vector.tensor_copy(vg[:, g, :, 0:128], srcA).then_inc(
                            s_vcA[c], 1
                        )
                    else:
                        vector.tensor_copy(
                            vg[:, g, 2:4, 0:128], srcA[:, 2:4, :]
                        ).then_inc(s_vcA[c], 1)
                        vector.tensor_copy(
                            vg[:, g, 0:2, 0:128], srcA[:, 0:2, :]
                        ).then_inc(s_vcA[c], 1)

                for c in range(C):
                    for g in range(4):
                        castA(c, g)

            @block.gpsimd
            def _(gpsimd):
                # identity stationary built before the stream arrives
                gpsimd.memset(idt[:], 1.0)
                gpsimd.affine_select(
                    out=idt[:],
                    in_=idt[:],
                    compare_op=mybir.AluOpType.is_equal,
                    fill=0.0,
                    base=0,
                    pattern=[[-1, 128]],
                    channel_multiplier=1,
                ).then_inc(s_id, 1)


            @block.scalar
            def _(scalar):
                # ACT-side warm-up (see DVE warm-up note)
                for w in range(ACT_WARMUPS):
                    scalar.copy(ot[:, 0:512], ot[:, 512:1024])
                # channel 0's diag stationaries: built on ACT while the
                # first image streams in
                scalar.wait_ge(s_hs, 16)
                scalar.copy(hsf[:], hst)  # bf16 -> f32 scalars
                scalar.wait_ge(s_id, 1)
                for i in range(nk[0]):
                    op = scalar.mul(
                        hdt[:, i * 128 : (i + 1) * 128],
                        idt[:],
                        hsf[:, i : i + 1],
                    )
                op.then_inc(s_hd[0], 1)

                def castB(c):
                    # one cast covering the whole 4-image B strip
                    b0 = (c % 2) * XW
                    vg = vsb[:, b0 : b0 + XW].rearrange(
                        "p (g ph u) -> p g ph u", g=BPC, ph=STR
                    )
                    scalar.wait_ge(s_mm[c], 5)
                    if c >= 2:
                        scalar.wait_ge(s_mm[c - 2], 7)  # vsb WAR vs H(c-2)
                    srcB = vB[:, 0 : BPC * 12].rearrange(
                        "p (g u ph) -> p g ph u", g=BPC, ph=STR
                    )
                    scalar.copy(
                        vg[:, 0:BPC, :, 128:131], srcB
                    ).then_inc(s_vcB[c], 1)

                def emit_out(c):
                    # one full copy + one DMA per channel, after its H1
                    # closes (reading an hp2 bank mid-H1 would race the PE)
                    o0 = (c % 2) * 512
                    scalar.wait_ge(s_mm[c], 7)
                    if c >= 2:
                        scalar.wait_ge(s_od[c - 2], 16)  # ot slot WAR
                    scalar.copy(
                        ot[:, o0 : o0 + 512], hp2[:, o0 : o0 + 512]
                    ).then_inc(s_out[c], 1)
                    scalar.dma_start(
                        out[c][:, 0:512], ot[:, o0 : o0 + 512]
                    ).then_inc(s_od[c], 16)

                for c in range(C):
                    castB(c)
                    emit_out(c)

        # (Block.__exit__ already emits the final barrier; an explicit
        # all_engine_barrier here would re-add the skipped gpsimd drain)
    nc.finalize()
    return nc


def _build_graph_general():
    import concourse.tile as tile
    from concourse import mybir

    nc = _bacc()
    x = nc.dram_tensor("x", [C, 4, 128, XW], mybir.dt.bfloat16, kind="ExternalInput")
    ep = nc.dram_tensor("ep", [128, EPACK_COLS], mybir.dt.bfloat16, kind="ExternalInput")
    out = nc.dram_tensor("out", [BPC, C, OH, OW], mybir.dt.float32, kind="ExternalOutput")

    with tile.TileContext(nc) as tc:
        with (
            tc.tile_pool(name="const", bufs=1) as constp,
            tc.tile_pool(name="xin", bufs=4) as xin,
            tc.tile_pool(name="ps", bufs=2, space="PSUM") as psp,
            tc.tile_pool(name="ot", bufs=2) as otp,
        ):
            ept = constp.tile([128, EPACK_COLS], mybir.dt.bfloat16)
            nc.scalar.dma_start(ept[:], ep[:])
            for c in range(C):
                psum = psp.tile([128, BPC * OW], mybir.dt.float32)
                for k in range(4):
                    xt = xin.tile([128, XW], mybir.dt.bfloat16)
                    nc.sync.dma_start(xt[:], x[c, k])
                    xg = xt[:].rearrange("p (g w) -> p g w", g=BPC)
                    for j in range(KS):
                        ph, q = j % STR, j // STR
                        off = ph * NPH + q
                        rhs = xg[:, :, off : off + OW]
                        t = c * KS + j
                        lo = t * SLOT + 96 - 32 * k
                        lhsT = ept[:, lo : lo + 128]
                        nc.tensor.matmul(
                            psum[:],
                            lhsT,
                            rhs,
                            start=(k == 0 and j == 0),
                            stop=(k == 3 and j == KS - 1),
                        )
                o = otp.tile([128, BPC * OW], mybir.dt.float32)
                nc.vector.tensor_copy(o[:], psum[:])
                dst = out[:, c].rearrange("g y x -> y g x")
                nc.sync.dma_start(dst, o[:].rearrange("y (g x) -> y g x", g=BPC))
    nc.finalize()
    return nc


def _decompose(weight):
    """Per-channel SVD; return (v[c,13], h[c,13]) if rank-1, else None."""
    vs, hs = [], []
    for c in range(C):
        w = weight[c, 0].astype(np.float64)
        u, s, vt = np.linalg.svd(w)
        if s[1] > 1e-5 * s[0]:
            return None
        sc = np.sqrt(s[0])
        vs.append(u[:, 0] * sc)
        hs.append(vt[0] * sc)
    return np.stack(vs), np.stack(hs)


def _pad_shard(inp):
    """[32,3,512,512] f32 -> [core, c, 128, k*img*524] fp8 (padded cols).

    fp8_e4m3 with error diffusion down image rows: quantization error of
    row r is carried into row r+1 before quantizing it, so the 13-tap
    vertical Gaussian sum sees anticorrelated errors (~2x lower output
    error than round-to-nearest fp8).
    """
    e4 = ml_dtypes.float8_e4m3
    pad = np.zeros((B, C, H, PW), np.float32)
    pad[..., PAD : PAD + W] = inp
    q = np.empty((B, C, H, PW), e4)
    carry = np.zeros((B, C, PW), np.float32)
    for r in range(H):
        t = pad[:, :, r, :] + carry
        qr = t.astype(e4)
        carry = t - qr.astype(np.float32)
        q[:, :, r, :] = qr
    arr = q.reshape(N_CORES, BPC, C, 4, 128, PW)
    arr = arr.transpose(0, 2, 4, 1, 3, 5).reshape(N_CORES, C, 128, 4 * XW)
    return np.ascontiguousarray(arr)


def _phase_shard(inp):
    """[32,3,512,512] f32 -> padded + phase-deinterleaved shards (general)."""
    bf16 = ml_dtypes.bfloat16
    pad = np.zeros((B, C, H, PW), np.float32)
    pad[..., PAD : PAD + W] = inp
    phmat = pad.reshape(B, C, H, NPH, STR).transpose(0, 1, 2, 4, 3)
    arr = phmat.reshape(N_CORES, BPC, C, 4, 128, STR, NPH)
    arr = arr.transpose(0, 2, 3, 4, 1, 5, 6).reshape(N_CORES, C, 4, 128, XW)
    return np.ascontiguousarray(arr).astype(bf16)


def _prep_rank1(inp, v, h):
    bf16 = ml_dtypes.bfloat16
    e4 = ml_dtypes.float8_e4m3
    arr = _pad_shard(inp)
    # fp8 chunk stationaries (DoubleRow matmuls need both operands fp8);
    # v-hat is chosen on the fp8 grid to minimize the rank-1 kernel
    # residual, with the compensating rescale folded into the h taps
    st = np.zeros((C, 4, 128, 128), np.float32)
    hadj = np.zeros((C, KS), np.float64)
    rr = np.arange(128)[:, None]   # contraction row within chunk
    mo = np.arange(128)[None, :]   # output row
    for c in range(C):
        vq, h_scale = _opt_fp8_v(v[c])
        vqf = vq.astype(np.float64)
        hadj[c] = np.asarray(h[c], np.float64) * h_scale
        for k in range(4):
            tap = rr + 128 * k - 4 * mo + PAD
            m = (tap >= 0) & (tap < KS)
            E = np.zeros((128, 128), np.float32)
            E[m] = vqf[tap[m]]
            st[c, k] = E
    st = np.ascontiguousarray(
        st.transpose(2, 0, 1, 3).reshape(128, C * STCOLS)
    ).astype(e4)
    # adaptive tap dropping: discard the largest set of horizontal taps
    # whose combined L2 mass is <= 5e-3 of the tap vector's norm (adds
    # ~3e-3 output rel err, far under the fp8 noise floor already present)
    keeps = []
    for c in range(C):
        a = np.abs(hadj[c])
        order = np.argsort(a)
        csq = np.cumsum(a[order] ** 2)
        ndrop = int(np.searchsorted(csq, (5e-3 * np.linalg.norm(hadj[c])) ** 2, "right"))
        ndrop = min(ndrop, KS - 1)  # always keep at least one tap
        keep = tuple(sorted(order[ndrop:].tolist()))
        keeps.append(keep)
    keeps = tuple(keeps)
    # kept h taps, replicated down 128 partitions: the device scales an
    # on-device identity by these to form the diag H stationaries
    nk = [len(k) for k in keeps]
    hsm = np.zeros((128, HS), np.float32)
    o = 0
    for c in range(C):
        for j in keeps[c]:
            hsm[:, o] = hadj[c, j]
            o += 1
    hsm = hsm.astype(bf16)
    # channels 1+2's diag H stationaries ride the DMA ring mid-stream
    hdm = np.zeros((128, (nk[1] + nk[2]) * 128), np.float32)
    idx = np.arange(128)
    o = 0
    for c in (1, 2):
        for j in keeps[c]:
            hdm[idx, o + idx] = hadj[c, j]
            o += 128
    hdm = hdm.astype(bf16)
    # h taps (bf16 bytes) + fp8 stationaries ride in front of the stream
    prefix = np.concatenate(
        [np.ascontiguousarray(hsm).view(np.uint8),
         np.ascontiguousarray(st).view(np.uint8)], axis=1
    )
    return keeps, [
        {
            "x": np.ascontiguousarray(
                np.concatenate(
                    [prefix,
                     arr[core].transpose(1, 0, 2)
                     .reshape(128, 3 * 4 * XW).view(np.uint8)],
                    axis=1,
                )
            ).view(e4),
            "hd": hdm,
        }
        for core in range(N_CORES)
    ]


def _prep_general(inp, weight):
    bf16 = ml_dtypes.bfloat16
    arr = _phase_shard(inp)
    epk = np.zeros((128, EPACK_COLS), np.float32)
    r = np.arange(128)
    for c in range(C):
        for j in range(KS):
            t = c * KS + j
            for s in range(-2, 34):
                i = r - 4 * s + PAD
                m = (i >= 0) & (i < KS)
                if m.any():
                    epk[m, t * SLOT + 96 + s] = weight[c, 0, i[m], j]
    epk = epk.astype(bf16)
    return [{"x": arr[core], "ep": epk} for core in range(N_CORES)]


def _prep(inp, weight):
    """Returns (graph_key, in_maps)."""
    inp = np.asarray(inp, dtype=np.float32)
    weight = np.asarray(weight, dtype=np.float32)
    vh = _decompose(weight)
    if vh is not None:
        keeps, in_maps = _prep_rank1(inp, *vh)
        return ("rank1", keeps), in_maps
    return "general", _prep_general(inp, weight)


def _graph(key):
    if key not in _CACHE:
        if key == "general":
            _CACHE[key] = _build_graph_general()
        else:
            _CACHE[key] = _build_graph_rank1_raw(key[1])
    return _CACHE[key]


def _run(key, in_maps):
    from concourse.bass_utils import run_bass_kernel_spmd

    nc = _graph(key)
    res = run_bass_kernel_spmd(nc, in_maps, core_ids=list(range(N_CORES)))
    outs = []
    for i in range(N_CORES):
        o = np.asarray(res.results[i]["out"])
        if o.ndim == 3:  # rank1 layout [C, OH, BPC*OW] -> [BPC, C, OH, OW]
            o = o.reshape(C, OH, BPC, OW).transpose(2, 0, 1, 3)
        outs.append(o)
    return np.concatenate(outs, axis=0).astype(np.float32)


def kernel(inp, weight):
    inp = np.asarray(inp, dtype=np.float32)
    weight = np.asarray(weight, dtype=np.float32)
    try:
        # host prep inside the try: an exotic rank-1 weight that breaks
        # the fp8 stationary build must also fall back, not crash
        key, in_maps = _prep(inp, weight)
        return _run(key, in_maps)
    except Exception:
        # fall back to the general (weight-agnostic) graph
        return _run("general", _prep_general(inp, weight))
